# revision 12
# baseline (speedup 1.0000x reference)
"""Multi-head attention (Keras-style, relu-activated dense projections)
for Trainium2, SPMD across 8 NeuronCores.

Problem (full shapes):
    B, S, D, H = 4, 1024, 1024, 16 ; DH = 64
    qp = relu(q @ Wq + bq); kp = relu(k @ Wk + bk); vp = relu(v @ Wv + bv)
    per head h: scores = qh @ kh^T / 8 ; attn = softmax(scores)
    out = relu(concat_h(attn @ vh) @ Wo + bo)

Sharding: core c = (batch b = c//2, head-group g = c%2). Each core computes
the 8 heads of group g for batch b end-to-end and produces the partial
output projection  attn_out_g @ Wo[g*512:(g+1)*512, :]  (no bias / relu).
Host sums the two partials per batch (bf16), adds bo, applies relu.

v5 design notes:
  - x inputs and Wq/Wk/Wv in fp8 e4m3 (weights prescaled x16 on the host
    so they clear the e4m3 subnormal range; the 1/16 is folded into the
    ACT relu `scale` for Q/K and the DVE (mult,max) tensor_scalar for V).
  - projections run perf_mode=DoubleRow: the [P, dt, free] tile layout is
    exactly the [Ki, Ko=2, free] 3D AP DoubleRow wants, so each matmul
    contracts 256 rows at 2 fp8/cell/cycle.  dt-outer loops load each
    stationary operand once for both pc halves (halves LDWEIGHTS, which
    measured ~100ns serialized per matmul).
  - all DMA'd tensors host-packed so SBUF partition p reads contiguous
    DRAM rows; x packed pc-major so each S-half is one 4KB-row transfer
    and the Q/K relus split per half -> first exp starts ~6us earlier.
  - unit U = (pc query-chunk, hp head-pair), pc-major.  Scores of unit U
    are 8 ut-chunks (2 row-group-paired K=64 bf16 matmuls + exp each)
    with one filler chunk after every ut: projections, attn@V + Z of
    earlier units, output projection.  Keeps the PE dense so the HAM
    clock gate stays at 8/8.
  - softmax denominator: exp ut-tiles leaf-added pairwise on DVE as they
    appear; z_pre (two M=1 matmuls at partitions 0/32 -> copies -> masked
    K=33 broadcast matmul -> reciprocal) runs as early filler; z_fin is
    one [128,512] multiply.
  - attn@V gated per-ut on exp(U, ut), column-pair packed (M=64), bf16.
"""

import numpy as np
from contextlib import ExitStack

import ml_dtypes

import concourse.bass as bass
import concourse.mybir as mybir
import concourse.tile as tile
from concourse import bacc

BF16NP = ml_dtypes.bfloat16
F8NP = ml_dtypes.float8_e4m3fn
WSCALE = 16.0

# ---- constants (hardcoded per the contract; kernel.py must be self-contained)
B, S, D, H = 4, 1024, 1024, 16
DG = 512          # feature slice per core (8 heads)
DH = 64
P = 128
NCORES = 8
NJT = DG // P     # 4 feature tiles == head pairs
NST = S // P      # 8 sequence tiles
NDT = D // P      # 8 contraction tiles for projections
NPC = S // 512    # 2 query chunks of 512

F32 = mybir.dt.float32
BF16 = mybir.dt.bfloat16
F8 = mybir.dt.float8e4
AF = mybir.ActivationFunctionType
ALU = mybir.AluOpType
DR = mybir.MatmulPerfMode.DoubleRow


def build_bass():
    nc = bacc.Bacc("TRN2", target_bir_lowering=False, debug=False,
                   num_devices=NCORES)

    # host-packed layouts: partition-major, contiguous free dim.
    # x: [p, pc, dt, s'] so one S-half is a contiguous 4KB row segment.
    xqT = nc.dram_tensor("xqT", [P, NPC, NDT, 512], F8,
                         kind="ExternalInput").ap()
    xkT = nc.dram_tensor("xkT", [P, NPC, NDT, 512], F8,
                         kind="ExternalInput").ap()
    xvT = nc.dram_tensor("xvT", [P, NPC, NDT, 512], F8,
                         kind="ExternalInput").ap()
    wq = nc.dram_tensor("wq", [P, NDT, DG], F8, kind="ExternalInput").ap()
    wk = nc.dram_tensor("wk", [P, NDT, DG], F8, kind="ExternalInput").ap()
    wv = nc.dram_tensor("wv", [P, NDT, DG], F8, kind="ExternalInput").ap()
    wo = nc.dram_tensor("wo", [P, NJT, D], BF16, kind="ExternalInput").ap()
    bq = nc.dram_tensor("bq", [P, NJT], F32, kind="ExternalInput").ap()
    bk = nc.dram_tensor("bk", [P, NJT], F32, kind="ExternalInput").ap()
    bv = nc.dram_tensor("bv", [1, DG], BF16, kind="ExternalInput").ap()
    bcm_in = nc.dram_tensor("bcmask", [33, P], BF16, kind="ExternalInput").ap()
    out = nc.dram_tensor("out", [S, D], BF16, kind="ExternalOutput").ap()

    # unit order: pc-major so the pc=0 output projection can start while
    # pc=1 attention is still draining.
    UNITS = [(pc, hp) for pc in range(NPC) for hp in range(NJT)]

    with tile.TileContext(nc) as tc, ExitStack() as ctx, \
            nc.allow_low_precision(reason="fp8/bf16 compute is intentional"):
        consts = ctx.enter_context(tc.tile_pool(name="consts", bufs=1))
        xpool = ctx.enter_context(tc.tile_pool(name="xpool", bufs=3))
        epool = ctx.enter_context(tc.tile_pool(name="epool", bufs=6))
        wpool = ctx.enter_context(tc.tile_pool(name="wpool", bufs=3))
        wopool = ctx.enter_context(tc.tile_pool(name="wopool", bufs=1))
        qkpool = ctx.enter_context(tc.tile_pool(name="qkpool", bufs=1))
        vpool = ctx.enter_context(tc.tile_pool(name="vpool", bufs=1))
        t1pool = ctx.enter_context(tc.tile_pool(name="t1pool", bufs=1))
        espool = ctx.enter_context(tc.tile_pool(name="espool", bufs=6))
        rpool = ctx.enter_context(tc.tile_pool(name="rpool", bufs=2))
        aopool = ctx.enter_context(tc.tile_pool(name="aopool", bufs=1))
        outpool = ctx.enter_context(tc.tile_pool(name="outpool", bufs=2))

        psA = ctx.enter_context(tc.tile_pool(name="psA", bufs=3, space="PSUM"))
        psB = ctx.enter_context(tc.tile_pool(name="psB", bufs=1, space="PSUM"))
        psZ = ctx.enter_context(tc.tile_pool(name="psZ", bufs=1, space="PSUM"))

        # --- big input DMAs first (the first matmuls gate on them); small
        # const DMAs go after wq..xk so they don't head-block the queue.
        def load_x(pool, dram, tag):
            t = pool.tile([P, NPC, NDT, 512], F8, tag=tag, name=tag)
            for pc in range(NPC):
                nc.sync.dma_start(out=t[:, pc], in_=dram[:, pc])
            return t

        def load3(pool, dram, shape, dt, tag):
            t = pool.tile(shape, dt, tag=tag, name=tag)
            nc.sync.dma_start(out=t, in_=dram)
            return t

        wqs = load3(wpool, wq, [P, NDT, DG], F8, "w")
        bqT = consts.tile([P, NJT], F32, tag="bqT")
        nc.sync.dma_start(out=bqT, in_=bq)
        bkT = consts.tile([P, NJT], F32, tag="bkT")
        nc.sync.dma_start(out=bkT, in_=bk)
        xq = load_x(xpool, xqT, "x")
        wks = load3(wpool, wk, [P, NDT, DG], F8, "w")
        xk = load_x(xpool, xkT, "x")

        bv_sb = consts.tile([1, DG], BF16, tag="bv")
        nc.sync.dma_start(out=bv_sb, in_=bv)
        bcmask = consts.tile([33, P], BF16, tag="bcmask")
        nc.sync.dma_start(out=bcmask, in_=bcm_in)
        onescol = consts.tile([P, 1], BF16, tag="onescol")
        nc.vector.memset(onescol, 1.0)
        onesrow = consts.tile([1, P], BF16, tag="onesrow")
        nc.vector.memset(onesrow, 1.0)
        # zsb: persistent Z staging rows (0 and 32); fill once with finite
        # values so the masked K=33 broadcast matmul never reads NaNs.
        zsb = consts.tile([33, 512], BF16, tag="zsb")
        nc.vector.memset(zsb, 1.0)

        wvs = load3(wpool, wv, [P, NDT, DG], F8, "w")
        xv = load_x(xpool, xvT, "x")
        wo3 = load3(wopool, wo, [P, NJT, D], BF16, "wo3")

        qpT = qkpool.tile([P, NJT, S], BF16, tag="qpT")
        kpT = qkpool.tile([P, NJT, S], BF16, tag="kpT")
        vpa = vpool.tile([P, NST, DG], BF16, tag="vpa")
        aoT3 = aopool.tile([P, NJT, S], BF16, tag="aoT3")

        # ---- chunk emitters ------------------------------------------------
        qk_ps = {}

        def qk_sub(which, hp, pc):
            """One pc-half of qpT/kpT pair hp: 4 DoubleRow matmuls (K=256
            each).  dt-outer/pc-inner keeps each stationary operand loaded
            for both halves; relu+bias+unscale per half."""
            xt, wt, bT, dst = ((xq, wqs, bqT, qpT) if which == "q"
                              else (xk, wks, bkT, kpT))
            if pc == 0:
                qk_ps[which] = psA.tile([P, 1024], F32, tag="ps", name="ps")
            ps = qk_ps[which]
            half = ps[:, pc * 512:(pc + 1) * 512]
            for dp in range(NDT // 2):
                nc.tensor.matmul(
                    half,
                    lhsT=wt[:, 2 * dp:2 * dp + 2, hp * P:(hp + 1) * P],
                    rhs=xt[:, pc, 2 * dp:2 * dp + 2, :],
                    start=(dp == 0), stop=(dp == NDT // 2 - 1),
                    perf_mode=DR, skip_group_check=True)
            if pc == 1:
                if which == "q":
                    # DVE relu: max(ps + 16*bq, 0) leaves qpT scaled x16;
                    # compensated in the exp scale (0.125/16).
                    nc.vector.tensor_scalar(dst[:, hp, :], ps,
                                            bT[:, hp:hp + 1], 0.0,
                                            ALU.add, ALU.max)
                else:
                    nc.scalar.activation(dst[:, hp, :], ps, AF.Relu,
                                         bias=bT[:, hp:hp + 1],
                                         scale=1.0 / WSCALE)

        def v_chunk(st):
            """V projection for sequence tile st: 4 DoubleRow matmuls +
            bias matmul (bv prescaled x16 on host), relu+unscale on DVE."""
            ps = psA.tile([P, 1024], F32, tag="ps", name="ps")
            half = ps[:, 0:512]
            pc, so = st // 4, (st % 4) * P
            for dp in range(NDT // 2):
                nc.tensor.matmul(
                    half,
                    lhsT=xv[:, pc, 2 * dp:2 * dp + 2, so:so + P],
                    rhs=wvs[:, 2 * dp:2 * dp + 2, :],
                    start=(dp == 0), stop=False,
                    perf_mode=DR, skip_group_check=True)
            nc.tensor.matmul(half, lhsT=onesrow, rhs=bv_sb,
                             start=False, stop=True, skip_group_check=True)
            nc.vector.tensor_scalar(vpa[:, st, :], half, 1.0 / WSCALE,
                                    0.0, ALU.mult, ALU.max)

        ex_tiles = {}
        lf_tiles = {}
        exsums = {}

        def sc_chunk(u, ut):
            """Scores ut-tile of unit u: 2 concurrent K=64 matmuls (head
            pair in row groups 0:64 / 64:128), exp, and the cascaded DVE
            leaf-sum toward the softmax denominator."""
            pc, hp = UNITS[u]
            if ut == 0:
                ex_tiles[u] = epool.tile([P, NST, 1024], BF16, tag="ex",
                                         name="ex")
            ex = ex_tiles[u]
            pslice = slice(pc * 512, (pc + 1) * 512)
            uslice = slice(ut * P, (ut + 1) * P)
            pw = psA.tile([P, 1024], F32, tag="ps", name="pw")
            nc.tensor.matmul(
                pw[:, 0:512],
                lhsT=kpT[0:DH, hp, uslice],
                rhs=qpT[0:DH, hp, pslice],
                start=True, stop=True)
            nc.tensor.matmul(
                pw[:, 512:1024],
                lhsT=kpT[DH:P, hp, uslice],
                rhs=qpT[DH:P, hp, pslice],
                start=True, stop=True)
            nc.scalar.activation(ex[:, ut, :], pw, AF.Exp, scale=0.125 / WSCALE)
            if ut % 2 == 1:
                j = ut // 2
                if j == 0:
                    lf_tiles[u] = t1pool.tile([P, 2, 1024], BF16, tag="lf",
                                              name="lf")
                lf = lf_tiles[u]
                dst = lf[:, 0, :] if j == 0 else lf[:, 1, :]
                nc.vector.tensor_add(dst, ex[:, ut - 1, :], ex[:, ut, :])
                if j in (1, 2):
                    nc.vector.tensor_add(lf[:, 0, :], lf[:, 0, :], lf[:, 1, :])
                elif j == 3:
                    exsum = espool.tile([P, 1024], BF16, tag="exsum",
                                        name="exsum")
                    nc.vector.tensor_add(exsum, lf[:, 0, :], lf[:, 1, :])
                    exsums[u] = exsum

        nt_tiles = {}

        def av_chunk(u, uts):
            """attn@V for unit u over ut in uts: column-pair-packed bf16
            matmuls accumulating into nt (head A rows 0:64, head B 64:128)."""
            pc, hp = UNITS[u]
            hA, hB = 2 * hp, 2 * hp + 1
            ex = ex_tiles[u]
            if uts[0] == 0:
                nt_tiles[u] = psB.tile([P, 512], F32, tag="nt", name="nt")
            nt = nt_tiles[u]
            for ut in uts:
                nc.tensor.matmul(
                    nt[0:DH, :],
                    lhsT=vpa[:, ut, hA * DH:(hA + 1) * DH],
                    rhs=ex[:, ut, 0:512],
                    start=(ut == 0), stop=(ut == NST - 1),
                    skip_group_check=True)
                nc.tensor.matmul(
                    nt[DH:P, :],
                    lhsT=vpa[:, ut, hB * DH:(hB + 1) * DH],
                    rhs=ex[:, ut, 512:1024],
                    start=(ut == 0), stop=(ut == NST - 1),
                    skip_group_check=True)

        rcps = {}

        def z_pre(u):
            """Z reduction + broadcast + reciprocal for unit u (no nt dep,
            so it can run as early filler)."""
            exsum = exsums.pop(u)
            zps = psZ.tile([P, 512], F32, tag="z", name="zps")
            nc.tensor.matmul(zps[0:1, :], lhsT=onescol,
                             rhs=exsum[:, 0:512], start=True, stop=True)
            nc.tensor.matmul(zps[32:33, :], lhsT=onescol,
                             rhs=exsum[:, 512:1024], start=True, stop=True)
            nc.vector.tensor_copy(zsb[0:1, :], zps[0:1, :])
            nc.vector.tensor_copy(zsb[32:33, :], zps[32:33, :])
            zbc = psZ.tile([P, 512], F32, tag="z", name="zbc")
            nc.tensor.matmul(zbc, lhsT=bcmask, rhs=zsb, start=True, stop=True)
            rcp = rpool.tile([P, 512], F32, tag="rcp", name="rcp")
            nc.vector.reciprocal_approx_fast(rcp, zbc)
            rcps[u] = rcp

        def z_fin(u):
            """Normalize attn@V of unit u into aoT3 (one DVE multiply)."""
            pc, hp = UNITS[u]
            pslice = slice(pc * 512, (pc + 1) * 512)
            nc.vector.tensor_mul(aoT3[:, hp, pslice], nt_tiles.pop(u),
                                 rcps.pop(u))
            del ex_tiles[u]

        def outp_chunk(pt, pool=None, drain_on_act=False):
            """Output projection for query tile pt: hp-outer 2x4
            accumulating matmuls, one wide copy, one DMA."""
            po = (pool or psA).tile([P, 1024], F32, tag="ps", name="po")
            for hp in range(NJT):
                for jj in range(2):
                    nc.tensor.matmul(
                        po[:, jj * 512:(jj + 1) * 512],
                        lhsT=aoT3[:, hp, pt * P:(pt + 1) * P],
                        rhs=wo3[:, hp, jj * 512:(jj + 1) * 512],
                        start=(hp == 0), stop=(hp == NJT - 1),
                        skip_group_check=True)
            os_ = outpool.tile([P, 1024], BF16, tag="os", name="os")
            if drain_on_act:
                nc.scalar.copy(os_, po)
            else:
                nc.vector.tensor_copy(os_, po)
            nc.sync.dma_start(out=out[pt * P:(pt + 1) * P, :], in_=os_)

        # ---- emission schedule --------------------------------------------
        def stretch(u, fillers):
            """Scores unit u (8 ut chunks) with one filler chunk after
            every ut."""
            fi = iter(fillers)
            for ut in range(NST):
                sc_chunk(u, ut)
                f = next(fi, None)
                if f is not None:
                    f()
            for f in fi:
                f()

        def qk4(hp):
            return [lambda w=w, pc=pc: qk_sub(w, hp, pc)
                    for w in ("q", "k") for pc in range(2)]

        def v4(st0):
            return [lambda st=st: v_chunk(st) for st in range(st0, st0 + 4)]

        def drain6(u):
            return [lambda: z_pre(u),
                    lambda: av_chunk(u, (0, 1)), lambda: av_chunk(u, (2, 3)),
                    lambda: av_chunk(u, (4, 5)), lambda: av_chunk(u, (6, 7)),
                    lambda: z_fin(u)]

        # head-pair 0 projections first (nothing to overlap them with)
        for f in qk4(0):
            f()

        d = [drain6(u) for u in range(8)]
        # U0..U3 = pc0 units; U4..U7 = pc1 units
        stretch(0, qk4(1) + v4(0))
        stretch(1, qk4(2) + v4(4))
        stretch(2, qk4(3) + d[0][:4])
        stretch(3, d[0][4:] + d[1])
        stretch(4, d[2] + d[3][:2])
        stretch(5, d[3][2:] + d[4][:4])
        stretch(6, d[4][4:] + [lambda: outp_chunk(0), lambda: outp_chunk(1)]
                + d[5][:4])
        stretch(7, d[5][4:] + [lambda: outp_chunk(2), lambda: outp_chunk(3)]
                + d[6][:4])

        # tail: last drains zipped with the pc1 output projection.
        for f in d[6][4:]:
            f()
        for f in d[7]:
            f()
        outp_chunk(4, drain_on_act=True)
        outp_chunk(5, pool=psA)
        outp_chunk(6, drain_on_act=True)
        outp_chunk(7, pool=psA)

    nc.compile()
    return nc


_CACHE = {}


def get_nc():
    if "nc" not in _CACHE:
        _CACHE["nc"] = build_bass()
    return _CACHE["nc"]


def make_bcmask():
    m = np.zeros((33, P), np.float32)
    m[0, 0:DH] = 1.0
    m[32, DH:P] = 1.0
    return m.astype(BF16NP)


def pack_rows(a, nd):
    """[nd*128, N] -> [128, nd, N] partition-major contiguous."""
    n = a.shape[1]
    return np.ascontiguousarray(a.reshape(nd, P, n).transpose(1, 0, 2))


def pack_x(a):
    """[1024 d, 1024 s] -> [128 p, 2 pc, 8 dt, 512] contiguous."""
    return np.ascontiguousarray(
        a.reshape(NDT, P, NPC, 512).transpose(1, 2, 0, 3))


def make_in_maps(q, k, v, Wq, bq, Wk, bk, Wv, bv, Wo, bo):
    q = np.asarray(q, np.float32)
    k = np.asarray(k, np.float32)
    v = np.asarray(v, np.float32)
    Wq = np.asarray(Wq, np.float32) * WSCALE
    Wk = np.asarray(Wk, np.float32) * WSCALE
    Wv = np.asarray(Wv, np.float32) * WSCALE
    Wo = np.asarray(Wo, np.float32)
    bq = np.asarray(bq, np.float32) * WSCALE
    bk = np.asarray(bk, np.float32)
    bv = np.asarray(bv, np.float32) * WSCALE

    qT = [pack_x(q[b].T.astype(F8NP)) for b in range(B)]
    kT = [pack_x(k[b].T.astype(F8NP)) for b in range(B)]
    vT = [pack_x(v[b].T.astype(F8NP)) for b in range(B)]
    bcm = make_bcmask()

    in_maps = []
    for c in range(NCORES):
        b, g = divmod(c, 2)
        sl = slice(g * DG, (g + 1) * DG)
        in_maps.append({
            "xqT": qT[b],
            "xkT": kT[b],
            "xvT": vT[b],
            "wq": pack_rows(Wq[:, sl].astype(F8NP), NDT),
            "wk": pack_rows(Wk[:, sl].astype(F8NP), NDT),
            "wv": pack_rows(Wv[:, sl].astype(F8NP), NDT),
            "bq": np.ascontiguousarray(bq[sl]).reshape(NJT, P).T.copy(),
            "bk": np.ascontiguousarray(bk[sl]).reshape(NJT, P).T.copy(),
            "bv": np.ascontiguousarray(bv[sl]).reshape(1, DG).astype(BF16NP),
            "wo": pack_rows(Wo[sl, :].astype(BF16NP), NJT),
            "bcmask": bcm,
        })
    return in_maps


def combine_outputs(parts, bo):
    bo = np.asarray(bo, np.float32)
    out = np.empty((B, S, D), np.float32)
    for b in range(B):
        out[b] = np.maximum(
            np.asarray(parts[2 * b], np.float32)
            + np.asarray(parts[2 * b + 1], np.float32) + bo[None, :], 0.0)
    return out


def run(in_maps, trace=False, **kwargs):
    from concourse.bass_utils import run_bass_kernel_spmd
    nc = get_nc()
    return run_bass_kernel_spmd(nc, in_maps, list(range(NCORES)),
                                trace=trace, **kwargs)


def kernel(q, k, v, Wq, bq, Wk, bk, Wv, bv, Wo, bo):
    in_maps = make_in_maps(q, k, v, Wq, bq, Wk, bk, Wv, bv, Wo, bo)
    res = run(in_maps)
    parts = [res.results[c]["out"] for c in range(NCORES)]
    return combine_outputs(parts, bo)


# revision 13
# speedup vs baseline: 1.0298x; 1.0298x over previous
"""Multi-head attention (Keras-style, relu-activated dense projections)
for Trainium2, SPMD across 8 NeuronCores.

Problem (full shapes):
    B, S, D, H = 4, 1024, 1024, 16 ; DH = 64
    qp = relu(q @ Wq + bq); kp = relu(k @ Wk + bk); vp = relu(v @ Wv + bv)
    per head h: scores = qh @ kh^T / 8 ; attn = softmax(scores)
    out = relu(concat_h(attn @ vh) @ Wo + bo)

Sharding: core c = (batch b = c//2, head-group g = c%2). Each core computes
the 8 heads of group g for batch b end-to-end and produces the partial
output projection  attn_out_g @ Wo[g*512:(g+1)*512, :]  (no bias / relu).
Host sums the two partials per batch (bf16), adds bo, applies relu.

v5 design notes:
  - x inputs and Wq/Wk/Wv in fp8 e4m3 (weights prescaled x16 on the host
    so they clear the e4m3 subnormal range; the 1/16 is folded into the
    ACT relu `scale` for Q/K and the DVE (mult,max) tensor_scalar for V).
  - projections run perf_mode=DoubleRow: the [P, dt, free] tile layout is
    exactly the [Ki, Ko=2, free] 3D AP DoubleRow wants, so each matmul
    contracts 256 rows at 2 fp8/cell/cycle.  dt-outer loops load each
    stationary operand once for both pc halves (halves LDWEIGHTS, which
    measured ~100ns serialized per matmul).
  - all DMA'd tensors host-packed so SBUF partition p reads contiguous
    DRAM rows; x packed pc-major so each S-half is one 4KB-row transfer
    and the Q/K relus split per half -> first exp starts ~6us earlier.
  - unit U = (pc query-chunk, hp head-pair), pc-major.  Scores of unit U
    are 8 ut-chunks (2 row-group-paired K=64 bf16 matmuls + exp each)
    with one filler chunk after every ut: projections, attn@V + Z of
    earlier units, output projection.  Keeps the PE dense so the HAM
    clock gate stays at 8/8.
  - softmax denominator: exp ut-tiles leaf-added pairwise on DVE as they
    appear; z_pre (two M=1 matmuls at partitions 0/32 -> copies -> masked
    K=33 broadcast matmul -> reciprocal) runs as early filler; z_fin is
    one [128,512] multiply.
  - attn@V gated per-ut on exp(U, ut), column-pair packed (M=64), bf16.
"""

import numpy as np
from contextlib import ExitStack

import ml_dtypes

import concourse.bass as bass
import concourse.mybir as mybir
import concourse.tile as tile
from concourse import bacc

BF16NP = ml_dtypes.bfloat16
F8NP = ml_dtypes.float8_e4m3fn
WSCALE = 16.0

# ---- constants (hardcoded per the contract; kernel.py must be self-contained)
B, S, D, H = 4, 1024, 1024, 16
DG = 512          # feature slice per core (8 heads)
DH = 64
P = 128
NCORES = 8
NJT = DG // P     # 4 feature tiles == head pairs
NST = S // P      # 8 sequence tiles
NDT = D // P      # 8 contraction tiles for projections
NPC = S // 512    # 2 query chunks of 512

F32 = mybir.dt.float32
BF16 = mybir.dt.bfloat16
F8 = mybir.dt.float8e4
AF = mybir.ActivationFunctionType
ALU = mybir.AluOpType
DR = mybir.MatmulPerfMode.DoubleRow


def build_bass():
    nc = bacc.Bacc("TRN2", target_bir_lowering=False, debug=False,
                   num_devices=NCORES)

    # host-packed layouts: partition-major, contiguous free dim.
    # x: [p, pc, dt, s'] so one S-half is a contiguous 4KB row segment.
    xqT = nc.dram_tensor("xqT", [P, NPC, NDT, 512], F8,
                         kind="ExternalInput").ap()
    xkT = nc.dram_tensor("xkT", [P, NPC, NDT, 512], F8,
                         kind="ExternalInput").ap()
    xvT = nc.dram_tensor("xvT", [P, NPC, NDT, 512], F8,
                         kind="ExternalInput").ap()
    wq = nc.dram_tensor("wq", [P, NDT, DG], F8, kind="ExternalInput").ap()
    wk = nc.dram_tensor("wk", [P, NDT, DG], F8, kind="ExternalInput").ap()
    wv = nc.dram_tensor("wv", [P, NDT, DG], F8, kind="ExternalInput").ap()
    wo = nc.dram_tensor("wo", [P, NJT, D], BF16, kind="ExternalInput").ap()
    bq = nc.dram_tensor("bq", [P, NJT], F32, kind="ExternalInput").ap()
    bk = nc.dram_tensor("bk", [P, NJT], F32, kind="ExternalInput").ap()
    bv = nc.dram_tensor("bv", [1, DG], BF16, kind="ExternalInput").ap()
    bcm_in = nc.dram_tensor("bcmask", [33, P], BF16, kind="ExternalInput").ap()
    out = nc.dram_tensor("out", [S, D], BF16, kind="ExternalOutput").ap()

    # unit order: pc-major so the pc=0 output projection can start while
    # pc=1 attention is still draining.
    UNITS = [(pc, hp) for pc in range(NPC) for hp in range(NJT)]

    with tile.TileContext(nc) as tc, ExitStack() as ctx, \
            nc.allow_low_precision(reason="fp8/bf16 compute is intentional"):
        consts = ctx.enter_context(tc.tile_pool(name="consts", bufs=1))
        xpool = ctx.enter_context(tc.tile_pool(name="xpool", bufs=3))
        epool = ctx.enter_context(tc.tile_pool(name="epool", bufs=6))
        wpool = ctx.enter_context(tc.tile_pool(name="wpool", bufs=3))
        wopool = ctx.enter_context(tc.tile_pool(name="wopool", bufs=1))
        qkpool = ctx.enter_context(tc.tile_pool(name="qkpool", bufs=1))
        vpool = ctx.enter_context(tc.tile_pool(name="vpool", bufs=1))
        t1pool = ctx.enter_context(tc.tile_pool(name="t1pool", bufs=1))
        espool = ctx.enter_context(tc.tile_pool(name="espool", bufs=6))
        rpool = ctx.enter_context(tc.tile_pool(name="rpool", bufs=2))
        aopool = ctx.enter_context(tc.tile_pool(name="aopool", bufs=1))
        outpool = ctx.enter_context(tc.tile_pool(name="outpool", bufs=2))

        psA = ctx.enter_context(tc.tile_pool(name="psA", bufs=3, space="PSUM"))
        psB = ctx.enter_context(tc.tile_pool(name="psB", bufs=1, space="PSUM"))
        psZ = ctx.enter_context(tc.tile_pool(name="psZ", bufs=1, space="PSUM"))

        # --- big input DMAs first (the first matmuls gate on them); small
        # const DMAs go after wq..xk so they don't head-block the queue.
        def load_x(pool, dram, tag):
            t = pool.tile([P, NPC, NDT, 512], F8, tag=tag, name=tag)
            for pc in range(NPC):
                nc.sync.dma_start(out=t[:, pc], in_=dram[:, pc])
            return t

        def load3(pool, dram, shape, dt, tag):
            t = pool.tile(shape, dt, tag=tag, name=tag)
            nc.sync.dma_start(out=t, in_=dram)
            return t

        wqs = load3(wpool, wq, [P, NDT, DG], F8, "w")
        bqT = consts.tile([P, NJT], F32, tag="bqT")
        nc.sync.dma_start(out=bqT, in_=bq)
        bkT = consts.tile([P, NJT], F32, tag="bkT")
        nc.sync.dma_start(out=bkT, in_=bk)
        xq = load_x(xpool, xqT, "x")
        wks = load3(wpool, wk, [P, NDT, DG], F8, "w")
        xk = load_x(xpool, xkT, "x")

        bv_sb = consts.tile([1, DG], BF16, tag="bv")
        nc.sync.dma_start(out=bv_sb, in_=bv)
        bcmask = consts.tile([33, P], BF16, tag="bcmask")
        nc.sync.dma_start(out=bcmask, in_=bcm_in)
        onescol = consts.tile([P, 1], BF16, tag="onescol")
        nc.vector.memset(onescol, 1.0)
        onesrow = consts.tile([1, P], BF16, tag="onesrow")
        nc.vector.memset(onesrow, 1.0)
        # zsb: persistent Z staging rows (0 and 32); fill once with finite
        # values so the masked K=33 broadcast matmul never reads NaNs.
        zsb = consts.tile([33, 512], BF16, tag="zsb")
        nc.vector.memset(zsb, 1.0)

        wvs = load3(wpool, wv, [P, NDT, DG], F8, "w")
        xv = load_x(xpool, xvT, "x")
        wo3 = load3(wopool, wo, [P, NJT, D], BF16, "wo3")

        qpT = qkpool.tile([P, NJT, S], BF16, tag="qpT")
        kpT = qkpool.tile([P, NJT, S], BF16, tag="kpT")
        vpa = vpool.tile([P, NST, DG], BF16, tag="vpa")
        aoT3 = aopool.tile([P, NJT, S], BF16, tag="aoT3")

        # ---- chunk emitters ------------------------------------------------
        qk_ps = {}

        def qk_sub(which, hp, pc):
            """One pc-half of qpT/kpT pair hp: 4 DoubleRow matmuls (K=256
            each).  dt-outer/pc-inner keeps each stationary operand loaded
            for both halves; relu+bias+unscale per half."""
            xt, wt, bT, dst = ((xq, wqs, bqT, qpT) if which == "q"
                              else (xk, wks, bkT, kpT))
            if pc == 0:
                qk_ps[which] = psA.tile([P, 1024], F32, tag="ps", name="ps")
            ps = qk_ps[which]
            half = ps[:, pc * 512:(pc + 1) * 512]
            for dp in range(NDT // 2):
                nc.tensor.matmul(
                    half,
                    lhsT=wt[:, 2 * dp:2 * dp + 2, hp * P:(hp + 1) * P],
                    rhs=xt[:, pc, 2 * dp:2 * dp + 2, :],
                    start=(dp == 0), stop=(dp == NDT // 2 - 1),
                    perf_mode=DR, skip_group_check=True)
            if pc == 1:
                # DVE relu: max(ps + 16*b, 0) leaves qpT/kpT scaled x16
                # each; compensated in the exp scale (0.125/256).  Keeps
                # ACT exp-only on the critical path.
                nc.vector.tensor_scalar(dst[:, hp, :], ps,
                                        bT[:, hp:hp + 1], 0.0,
                                        ALU.add, ALU.max)

        def v_chunk(st):
            """V projection for sequence tile st: 4 DoubleRow matmuls +
            bias matmul (bv prescaled x16 on host), relu+unscale on DVE."""
            ps = psA.tile([P, 1024], F32, tag="ps", name="ps")
            half = ps[:, 0:512]
            pc, so = st // 4, (st % 4) * P
            for dp in range(NDT // 2):
                nc.tensor.matmul(
                    half,
                    lhsT=xv[:, pc, 2 * dp:2 * dp + 2, so:so + P],
                    rhs=wvs[:, 2 * dp:2 * dp + 2, :],
                    start=(dp == 0), stop=False,
                    perf_mode=DR, skip_group_check=True)
            nc.tensor.matmul(half, lhsT=onesrow, rhs=bv_sb,
                             start=False, stop=True, skip_group_check=True)
            nc.vector.tensor_scalar(vpa[:, st, :], half, 1.0 / WSCALE,
                                    0.0, ALU.mult, ALU.max)

        ex_tiles = {}
        lf_tiles = {}
        exsums = {}

        def sc_chunk(u, ut):
            """Scores ut-tile of unit u: 2 concurrent K=64 matmuls (head
            pair in row groups 0:64 / 64:128), exp, and the cascaded DVE
            leaf-sum toward the softmax denominator."""
            pc, hp = UNITS[u]
            if ut == 0:
                ex_tiles[u] = epool.tile([P, NST, 1024], BF16, tag="ex",
                                         name="ex")
            ex = ex_tiles[u]
            pslice = slice(pc * 512, (pc + 1) * 512)
            uslice = slice(ut * P, (ut + 1) * P)
            pw = psA.tile([P, 1024], F32, tag="ps", name="pw")
            nc.tensor.matmul(
                pw[:, 0:512],
                lhsT=kpT[0:DH, hp, uslice],
                rhs=qpT[0:DH, hp, pslice],
                start=True, stop=True)
            nc.tensor.matmul(
                pw[:, 512:1024],
                lhsT=kpT[DH:P, hp, uslice],
                rhs=qpT[DH:P, hp, pslice],
                start=True, stop=True)
            nc.scalar.activation(ex[:, ut, :], pw, AF.Exp,
                                 scale=0.125 / (WSCALE * WSCALE))
            if ut % 2 == 1:
                j = ut // 2
                if j == 0:
                    lf_tiles[u] = t1pool.tile([P, 2, 1024], BF16, tag="lf",
                                              name="lf")
                lf = lf_tiles[u]
                dst = lf[:, 0, :] if j == 0 else lf[:, 1, :]
                nc.vector.tensor_add(dst, ex[:, ut - 1, :], ex[:, ut, :])
                if j in (1, 2):
                    nc.vector.tensor_add(lf[:, 0, :], lf[:, 0, :], lf[:, 1, :])
                elif j == 3:
                    exsum = espool.tile([P, 1024], BF16, tag="exsum",
                                        name="exsum")
                    nc.vector.tensor_add(exsum, lf[:, 0, :], lf[:, 1, :])
                    exsums[u] = exsum

        nt_tiles = {}

        def av_chunk(u, uts):
            """attn@V for unit u over ut in uts: column-pair-packed bf16
            matmuls accumulating into nt (head A rows 0:64, head B 64:128)."""
            pc, hp = UNITS[u]
            hA, hB = 2 * hp, 2 * hp + 1
            ex = ex_tiles[u]
            if uts[0] == 0:
                nt_tiles[u] = psB.tile([P, 512], F32, tag="nt", name="nt")
            nt = nt_tiles[u]
            for ut in uts:
                nc.tensor.matmul(
                    nt[0:DH, :],
                    lhsT=vpa[:, ut, hA * DH:(hA + 1) * DH],
                    rhs=ex[:, ut, 0:512],
                    start=(ut == 0), stop=(ut == NST - 1),
                    skip_group_check=True)
                nc.tensor.matmul(
                    nt[DH:P, :],
                    lhsT=vpa[:, ut, hB * DH:(hB + 1) * DH],
                    rhs=ex[:, ut, 512:1024],
                    start=(ut == 0), stop=(ut == NST - 1),
                    skip_group_check=True)

        rcps = {}

        def z_pre(u):
            """Z reduction + broadcast + reciprocal for unit u (no nt dep,
            so it can run as early filler)."""
            exsum = exsums.pop(u)
            zps = psZ.tile([P, 512], F32, tag="z", name="zps")
            nc.tensor.matmul(zps[0:1, :], lhsT=onescol,
                             rhs=exsum[:, 0:512], start=True, stop=True)
            nc.tensor.matmul(zps[32:33, :], lhsT=onescol,
                             rhs=exsum[:, 512:1024], start=True, stop=True)
            nc.vector.tensor_copy(zsb[0:1, :], zps[0:1, :])
            nc.vector.tensor_copy(zsb[32:33, :], zps[32:33, :])
            zbc = psZ.tile([P, 512], F32, tag="z", name="zbc")
            nc.tensor.matmul(zbc, lhsT=bcmask, rhs=zsb, start=True, stop=True)
            rcp = rpool.tile([P, 512], F32, tag="rcp", name="rcp")
            nc.vector.reciprocal_approx_fast(rcp, zbc)
            rcps[u] = rcp

        def z_fin(u):
            """Normalize attn@V of unit u into aoT3 (one DVE multiply)."""
            pc, hp = UNITS[u]
            pslice = slice(pc * 512, (pc + 1) * 512)
            nc.vector.tensor_mul(aoT3[:, hp, pslice], nt_tiles.pop(u),
                                 rcps.pop(u))
            del ex_tiles[u]

        def outp_chunk(pt, pool=None, drain_on_act=False):
            """Output projection for query tile pt: hp-outer 2x4
            accumulating matmuls, one wide copy, one DMA."""
            po = (pool or psA).tile([P, 1024], F32, tag="ps", name="po")
            for hp in range(NJT):
                for jj in range(2):
                    nc.tensor.matmul(
                        po[:, jj * 512:(jj + 1) * 512],
                        lhsT=aoT3[:, hp, pt * P:(pt + 1) * P],
                        rhs=wo3[:, hp, jj * 512:(jj + 1) * 512],
                        start=(hp == 0), stop=(hp == NJT - 1),
                        skip_group_check=True)
            os_ = outpool.tile([P, 1024], BF16, tag="os", name="os")
            if drain_on_act:
                nc.scalar.copy(os_, po)
            else:
                nc.vector.tensor_copy(os_, po)
            nc.sync.dma_start(out=out[pt * P:(pt + 1) * P, :], in_=os_)

        # ---- emission schedule --------------------------------------------
        def stretch(u, fillers):
            """Scores unit u (8 ut chunks) with one filler chunk after
            every ut."""
            fi = iter(fillers)
            for ut in range(NST):
                sc_chunk(u, ut)
                f = next(fi, None)
                if f is not None:
                    f()
            for f in fi:
                f()

        def qk4(hp):
            return [lambda w=w, pc=pc: qk_sub(w, hp, pc)
                    for w in ("q", "k") for pc in range(2)]

        def v4(st0):
            return [lambda st=st: v_chunk(st) for st in range(st0, st0 + 4)]

        def drain6(u):
            return [lambda: z_pre(u),
                    lambda: av_chunk(u, (0, 1)), lambda: av_chunk(u, (2, 3)),
                    lambda: av_chunk(u, (4, 5)), lambda: av_chunk(u, (6, 7)),
                    lambda: z_fin(u)]

        # head-pair 0 projections first (nothing to overlap them with)
        for f in qk4(0):
            f()

        d = [drain6(u) for u in range(8)]
        # U0..U3 = pc0 units; U4..U7 = pc1 units
        stretch(0, qk4(1) + v4(0))
        stretch(1, qk4(2) + v4(4))
        stretch(2, qk4(3) + d[0][:4])
        stretch(3, d[0][4:] + d[1])
        stretch(4, d[2] + d[3][:2])
        stretch(5, d[3][2:] + d[4][:4])
        stretch(6, d[4][4:] + [lambda: outp_chunk(0), lambda: outp_chunk(1)]
                + d[5][:4])
        stretch(7, d[5][4:] + [lambda: outp_chunk(2), lambda: outp_chunk(3)]
                + d[6][:4])

        # tail: last drains zipped with the pc1 output projection.
        for f in d[6][4:]:
            f()
        for f in d[7]:
            f()
        outp_chunk(4, drain_on_act=True)
        outp_chunk(5, pool=psA)
        outp_chunk(6, drain_on_act=True)
        outp_chunk(7, pool=psA)

    nc.compile()
    return nc


_CACHE = {}


def get_nc():
    if "nc" not in _CACHE:
        _CACHE["nc"] = build_bass()
    return _CACHE["nc"]


def make_bcmask():
    m = np.zeros((33, P), np.float32)
    m[0, 0:DH] = 1.0
    m[32, DH:P] = 1.0
    return m.astype(BF16NP)


def pack_rows(a, nd):
    """[nd*128, N] -> [128, nd, N] partition-major contiguous."""
    n = a.shape[1]
    return np.ascontiguousarray(a.reshape(nd, P, n).transpose(1, 0, 2))


def pack_x(a):
    """[1024 d, 1024 s] -> [128 p, 2 pc, 8 dt, 512] contiguous."""
    return np.ascontiguousarray(
        a.reshape(NDT, P, NPC, 512).transpose(1, 2, 0, 3))


def make_in_maps(q, k, v, Wq, bq, Wk, bk, Wv, bv, Wo, bo):
    q = np.asarray(q, np.float32)
    k = np.asarray(k, np.float32)
    v = np.asarray(v, np.float32)
    Wq = np.asarray(Wq, np.float32) * WSCALE
    Wk = np.asarray(Wk, np.float32) * WSCALE
    Wv = np.asarray(Wv, np.float32) * WSCALE
    Wo = np.asarray(Wo, np.float32)
    bq = np.asarray(bq, np.float32) * WSCALE
    bk = np.asarray(bk, np.float32) * WSCALE
    bv = np.asarray(bv, np.float32) * WSCALE

    qT = [pack_x(q[b].T.astype(F8NP)) for b in range(B)]
    kT = [pack_x(k[b].T.astype(F8NP)) for b in range(B)]
    vT = [pack_x(v[b].T.astype(F8NP)) for b in range(B)]
    bcm = make_bcmask()

    in_maps = []
    for c in range(NCORES):
        b, g = divmod(c, 2)
        sl = slice(g * DG, (g + 1) * DG)
        in_maps.append({
            "xqT": qT[b],
            "xkT": kT[b],
            "xvT": vT[b],
            "wq": pack_rows(Wq[:, sl].astype(F8NP), NDT),
            "wk": pack_rows(Wk[:, sl].astype(F8NP), NDT),
            "wv": pack_rows(Wv[:, sl].astype(F8NP), NDT),
            "bq": np.ascontiguousarray(bq[sl]).reshape(NJT, P).T.copy(),
            "bk": np.ascontiguousarray(bk[sl]).reshape(NJT, P).T.copy(),
            "bv": np.ascontiguousarray(bv[sl]).reshape(1, DG).astype(BF16NP),
            "wo": pack_rows(Wo[sl, :].astype(BF16NP), NJT),
            "bcmask": bcm,
        })
    return in_maps


def combine_outputs(parts, bo):
    bo = np.asarray(bo, np.float32)
    out = np.empty((B, S, D), np.float32)
    for b in range(B):
        out[b] = np.maximum(
            np.asarray(parts[2 * b], np.float32)
            + np.asarray(parts[2 * b + 1], np.float32) + bo[None, :], 0.0)
    return out


def run(in_maps, trace=False, **kwargs):
    from concourse.bass_utils import run_bass_kernel_spmd
    nc = get_nc()
    return run_bass_kernel_spmd(nc, in_maps, list(range(NCORES)),
                                trace=trace, **kwargs)


def kernel(q, k, v, Wq, bq, Wk, bk, Wv, bv, Wo, bo):
    in_maps = make_in_maps(q, k, v, Wq, bq, Wk, bk, Wv, bv, Wo, bo)
    res = run(in_maps)
    parts = [res.results[c]["out"] for c in range(NCORES)]
    return combine_outputs(parts, bo)
